# revision 1
# baseline (speedup 1.0000x reference)
"""MeanNSE (segment-reduce) Trainium2 kernel — 8 NeuronCores, data-parallel.

Math per basin b in [0, 671), with t = y_true, d = y_true - y_pred:
  sum_t[b], sum_t2[b], sum_d2[b]  (device, fp32 PSUM accumulation)
  count[b]                        (host np.bincount)
  ss_tot = sum_t2 - sum_t^2/count   == sum((t - mean_b)^2) in one pass
  answer = mean_b(1 - sum_d2 / (ss_tot + 1e-10))

Device algorithm per core (E = N/8 elements):
  Radix-decompose b = q*42 + r (q in [0,16), r in [0,42)). Elements are
  processed in chunks of 128 (one per SBUF partition). For every chunk f:

      PSUM[42, 48] += Vcm[:, f, :].T @ SU[:, f, :]        (TensorE, bf16)

  where Vcm[k, f, r'] = [r_k == r'] is a one-hot of r and SU[k, f, s*16+q']
  = stat_s(k) * [q_k == q'] are stat-scaled one-hots of q
  (stats = {t, t^2, d^2}).

  Both one-hot tensors are built in chunk-major layout by GPSIMD
  `local_scatter` (zero + per-partition scatter into 2047-element blocks):
  ScalarE writes an interleaved bf16 stat stream data3[p, 3f+s], and the
  host supplies per-element int16 scatter indices, so the only per-element
  device compute is the scatter write itself. Chunk-major layout makes both
  matmul operands contiguous, which is what lets TensorE sustain its
  fastest LDWEIGHTS+MATMUL pace (strided operand APs cost 3-6x).

  The tiny per-core [42, 48] fp32 partials are combined on the host in
  float64. Rel. error vs the fp32 jax reference is ~5e-7 (bf16 stats are
  exact for the one-hots; products accumulate in fp32 PSUM).
"""

import sys

sys.path.insert(0, "/opt/trn_rl_repo")

import numpy as np
import ml_dtypes  # noqa: F401  (bf16 dtype availability)

import concourse.bacc as bacc
import concourse.mybir as mybir
import concourse.tile as tile
from concourse.bass_utils import run_bass_kernel_spmd

F32 = mybir.dt.float32
BF16 = mybir.dt.bfloat16
I16 = mybir.dt.int16

N_CORES = 8
N_TOTAL = 16777216
E = N_TOTAL // N_CORES
N_BASINS = 671
EPS = 1e-10

QW = 16  # q-digit width; b = q*42 + r
RW = 42  # r-digit width
NSTAT = 3
FB = 42  # SU scatter block: 42*48 = 2016 <= 2047 (remainder tiles use 32)
FB_V = 48  # V scatter block: 48*42 = 2016 <= 2047
F_TILE = 336  # chunks per tile (elements per partition per tile)

_AF = mybir.ActivationFunctionType

_cache = {}


def _build(E, F=F_TILE):
    n_main, rem = divmod(E, 128 * F)
    tile_sizes = [F] * n_main
    assert rem % 128 == 0
    if rem:
        tile_sizes.append(rem // 128)
    for Ft in tile_sizes:
        assert Ft % (FB if Ft % FB == 0 else 32) == 0

    nc = bacc.Bacc()
    yt = nc.declare_dram_parameter("yt", [E], F32, isOutput=False)
    yp = nc.declare_dram_parameter("yp", [E], F32, isOutput=False)
    vidx = nc.declare_dram_parameter("vidx", [E], I16, isOutput=False)
    uidx3 = nc.declare_dram_parameter("uidx3", [3 * E], I16, isOutput=False)
    out = nc.declare_dram_parameter("partial", [RW, NSTAT * QW], F32, isOutput=True)
    n_chunks = E // 128

    with tile.TileContext(nc) as tc:
        with (
            tc.tile_pool(name="cpool", bufs=1) as cpool,
            tc.tile_pool(name="psum", bufs=1, space="PSUM") as psum_pool,
            tc.tile_pool(name="io", bufs=3) as io_pool,
            tc.tile_pool(name="work", bufs=2) as work_pool,
        ):
            ones = cpool.tile([128, FB_V], BF16, tag="ones")
            nc.gpsimd.memset(ones[:, :], 1.0)
            acc = psum_pool.tile([RW, NSTAT * QW], F32)
            base = 0
            chunk_idx = 0
            for t, Ft in enumerate(tile_sizes):
                n_el = 128 * Ft
                sl = lambda x: x[base : base + n_el].rearrange(
                    "(p f) -> p f", p=128, f=Ft
                )
                tt_ = io_pool.tile([128, Ft], F32, tag="yt")
                tp_ = io_pool.tile([128, Ft], F32, tag="yp")
                tvi = io_pool.tile([128, Ft], I16, tag="vidx")
                tui = io_pool.tile([128, 3 * Ft], I16, tag="uidx3")
                nc.sync.dma_start(tt_[:, :], sl(yt))
                nc.sync.dma_start(tp_[:, :], sl(yp))
                nc.sync.dma_start(tvi[:, :], sl(vidx))
                nc.sync.dma_start(
                    tui[:, :],
                    uidx3[3 * base : 3 * (base + n_el)].rearrange(
                        "(p f) -> p f", p=128, f=3 * Ft
                    ),
                )

                # interleaved bf16 stats: data3[p, 3f+s] = {t, t^2, d^2}
                data3 = work_pool.tile([128, 3 * Ft], BF16, tag="data3")
                dtmp = work_pool.tile([128, Ft], F32, tag="dtmp")
                d3v = data3[:, :].rearrange("p (f s) -> p f s", s=3)
                nc.scalar.copy(d3v[:, :, 0], tt_[:, :])
                nc.scalar.square(d3v[:, :, 1], tt_[:, :])
                nc.vector.tensor_sub(dtmp[:, :], tt_[:, :], tp_[:, :])
                nc.scalar.square(d3v[:, :, 2], dtmp[:, :])

                SU = work_pool.tile([128, Ft, NSTAT * QW], BF16, tag="SU")
                Vcm = work_pool.tile([128, Ft, RW], BF16, tag="Vcm")
                fbu = FB if Ft % FB == 0 else 32
                for f0 in range(0, Ft, fbu):
                    nc.gpsimd.local_scatter(
                        SU[:, f0 : f0 + fbu, :].rearrange("p a b -> p (a b)"),
                        data3[:, 3 * f0 : 3 * (f0 + fbu)],
                        tui[:, 3 * f0 : 3 * (f0 + fbu)],
                        channels=128,
                        num_elems=fbu * NSTAT * QW,
                        num_idxs=3 * fbu,
                    )
                fbv = FB_V if Ft % FB_V == 0 else 32
                for f0 in range(0, Ft, fbv):
                    nc.gpsimd.local_scatter(
                        Vcm[:, f0 : f0 + fbv, :].rearrange("p a b -> p (a b)"),
                        ones[:, :fbv],
                        tvi[:, f0 : f0 + fbv],
                        channels=128,
                        num_elems=fbv * RW,
                        num_idxs=fbv,
                    )
                for f in range(Ft):
                    nc.tensor.matmul(
                        acc[:, :],
                        lhsT=Vcm[:, f, :],
                        rhs=SU[:, f, :],
                        start=(chunk_idx == 0),
                        stop=(chunk_idx == n_chunks - 1),
                    )
                    chunk_idx += 1
                base += n_el
            res = cpool.tile([RW, NSTAT * QW], F32, tag="res")
            nc.vector.tensor_copy(res[:, :], acc[:, :])
            nc.sync.dma_start(out[:, :], res[:, :])
    nc.compile()
    return nc


def _get_nc():
    if "nc" not in _cache:
        _cache["nc"] = _build(E)
    return _cache["nc"]


def _host_indices(basin_u16):
    """Scatter indices for the fixed [tile, partition, f] element layout."""
    q = (basin_u16 // RW).astype(np.int16)
    r = (basin_u16 % RW).astype(np.int16)
    n = len(basin_u16)
    fparts = []
    vfb = []
    ufb = []
    remaining = E
    while remaining > 0:
        Ft = F_TILE if remaining >= 128 * F_TILE else remaining // 128
        fparts.append(np.tile(np.arange(Ft, dtype=np.int16), 128))
        fbv = FB_V if Ft % FB_V == 0 else 32
        vfb.append(np.full(128 * Ft, fbv, np.int16))
        ufb.append(np.full(128 * Ft, FB if Ft % FB == 0 else 32, np.int16))
        remaining -= 128 * Ft
    fpos1 = np.concatenate(fparts)
    vfb1 = np.concatenate(vfb)
    ufb1 = np.concatenate(ufb)
    vidx = np.empty(n, np.int16)
    uidx3 = np.empty(3 * n, np.int16)
    s_off = np.array([0, QW, 2 * QW], np.int16)
    for c in range(n // E):
        seg = slice(c * E, (c + 1) * E)
        vidx[seg] = (fpos1 % vfb1) * RW + r[seg]
        base3 = (
            ((fpos1 % ufb1).astype(np.int32) * (NSTAT * QW))[:, None]
            + s_off[None, :]
            + q[seg][:, None]
        )
        uidx3[3 * c * E : 3 * (c + 1) * E] = base3.astype(np.int16).ravel()
    return vidx, uidx3


def kernel(y_pred, y_true, basin):
    y_pred = np.ascontiguousarray(np.asarray(y_pred, dtype=np.float32))
    y_true = np.ascontiguousarray(np.asarray(y_true, dtype=np.float32))
    b16 = np.asarray(basin).astype(np.uint16)
    vidx, uidx3 = _host_indices(b16)
    counts = np.bincount(b16, minlength=QW * RW)

    nc = _get_nc()
    in_maps = []
    for c in range(N_CORES):
        sl = slice(c * E, (c + 1) * E)
        in_maps.append(
            {
                "yt": y_true[sl],
                "yp": y_pred[sl],
                "vidx": vidx[sl],
                "uidx3": uidx3[3 * c * E : 3 * (c + 1) * E],
            }
        )
    res = run_bass_kernel_spmd(nc, in_maps, list(range(N_CORES)))

    tot = np.zeros((RW, NSTAT * QW), dtype=np.float64)
    for c in range(N_CORES):
        tot += res.results[c]["partial"].astype(np.float64)
    # psum[r, s*QW+q] -> [s, b] with b = q*RW + r
    tot = tot.reshape(RW, NSTAT, QW).transpose(1, 2, 0).reshape(NSTAT, QW * RW)
    cnt = counts[:N_BASINS].astype(np.float64)
    s_t = tot[0, :N_BASINS]
    s_t2 = tot[1, :N_BASINS]
    s_d2 = tot[2, :N_BASINS]
    ss_tot = s_t2 - s_t * s_t / cnt
    nse = 1.0 - s_d2 / (ss_tot + EPS)
    return np.float32(nse.mean())



# revision 2
# speedup vs baseline: 1.0064x; 1.0064x over previous
"""MeanNSE (segment-reduce) Trainium2 kernel — 8 NeuronCores, data-parallel.

Host side (index prep only, same spirit as np.bincount for counts):
  * counting-sort element indices by basin id (stable argsort on uint16),
  * zero-pad each basin's run to a multiple of G=128 and lay the padded
    stream out column-major so every 128-element group is one SBUF column:
    padded rank r -> (partition p = r%128, group q = r//128 -> core, tile,
    column). Groups never straddle tiles or cores, so every column of every
    device tile belongs to exactly one basin (zeros contribute nothing).

Device (per core, E = 2162688 padded elements as 11 bf16 tiles [128, 1536]):
  VectorE: d = t - p                        (tensor_sub, bf16)
  GPSIMD : z_t2 = t*t   (tiles 0-7)         (tensor_mul, bf16)
  VectorE: z_t2 = t*t   (tiles 8-10)        (load balance)
  ScalarE: z_d2 = d^2                       (activation Square, bf16)
  TensorE: all group reductions. For each 512-column chunk k (33 per core),
    an fp32-PSUM-accumulating matmul with one-hot weights oneh[:, k, :33]
    adds each column's 128-partition sum into PSUM row k (rows not selected
    by the one-hot receive +0), for each of the three streams t, z_t2, z_d2:
        ps_s[0:33, :512] (+)= onehot_k^T @ stream_s[:, chunk_k]
  3 PSUM->SBUF copies + 3 eviction DMAs [33, 512] fp32.

Host combine in float64: np.bincount of the per-group partials per basin;
ss_tot = S_t2 - S_t^2/count (one-pass identity, counts from np.bincount);
answer = mean(1 - S_d2/(ss_tot + 1e-10)). Overall rel err vs fp32
reference ~1.4e-4 (bf16 products, fp32 accumulation).
"""

import sys

sys.path.insert(0, "/opt/trn_rl_repo")

import numpy as np
import ml_dtypes  # noqa: F401

import concourse.bacc as bacc
import concourse.mybir as mybir
import concourse.tile as tile
from concourse.bass_utils import run_bass_kernel_spmd

F32 = mybir.dt.float32
F16 = mybir.dt.float16
BF16 = mybir.dt.bfloat16

N_CORES = 8
N_TOTAL = 16777216
N_BASINS = 671
EPS = 1e-10

G = 128  # elements per group (pad unit) = one SBUF column
F_T = 1536  # columns per tile
N_T = 11  # tiles per core
COLS = N_T * F_T  # 16896 groups (columns) per core
E = 128 * COLS  # 2162688 elements per core
E_TOT = N_CORES * E  # 17301504 >= 16777216 + 671*127 (max padding)
CHUNK = 512  # columns per PSUM row (fp32 capacity of one bank row)
NCH = COLS // CHUNK  # 33 chunks (PSUM rows) per core per stat

_AF = mybir.ActivationFunctionType
_ALU = mybir.AluOpType

_cache = {}


def _build():
    nc = bacc.Bacc()
    yt = nc.declare_dram_parameter("yt", [E], BF16, isOutput=False)
    yp = nc.declare_dram_parameter("yp", [E], BF16, isOutput=False)
    pt = nc.declare_dram_parameter("pt", [NCH, CHUNK], F32, isOutput=True)
    pt2 = nc.declare_dram_parameter("pt2", [NCH, CHUNK], F32, isOutput=True)
    pd2 = nc.declare_dram_parameter("pd2", [NCH, CHUNK], F32, isOutput=True)

    with tile.TileContext(nc) as tc:
        with (
            tc.tile_pool(name="const", bufs=1) as cpool,
            tc.tile_pool(name="io", bufs=4) as io_pool,
            tc.tile_pool(name="work", bufs=4) as work_pool,
            tc.tile_pool(name="psum", bufs=1, space="PSUM") as psum_pool,
        ):
            # one-hot weight rows: oneh[p, k, m] = (m == k), same per partition
            oneh = cpool.tile([128, NCH, NCH], BF16, tag="oneh")
            nc.gpsimd.memset(oneh[:, :, :], 0.0)
            for k in range(NCH):
                nc.gpsimd.memset(oneh[:, k, k : k + 1], 1.0)
            ps_t = psum_pool.tile([128, CHUNK], F32, tag="ps_t")
            ps_t2 = psum_pool.tile([128, CHUNK], F32, tag="ps_t2")
            ps_d2 = psum_pool.tile([128, CHUNK], F32, tag="ps_d2")
            for t in range(N_T):
                base = t * 128 * F_T
                tt = io_pool.tile([128, F_T], BF16, tag="yt")
                tp = io_pool.tile([128, F_T], BF16, tag="yp")
                nc.sync.dma_start(
                    tt[:, :],
                    yt[base : base + 128 * F_T].rearrange("(p f) -> p f", p=128),
                )
                nc.sync.dma_start(
                    tp[:, :],
                    yp[base : base + 128 * F_T].rearrange("(p f) -> p f", p=128),
                )
                d = work_pool.tile([128, F_T], BF16, tag="d")
                zt2 = work_pool.tile([128, F_T], BF16, tag="zt2")
                zd2 = work_pool.tile([128, F_T], BF16, tag="zd2")
                nc.vector.tensor_sub(d[:, :], tt[:, :], tp[:, :])
                nc.scalar.square(zd2[:, :], d[:, :])
                if t < 8:
                    nc.gpsimd.tensor_mul(zt2[:, :], tt[:, :], tt[:, :])
                else:
                    nc.vector.tensor_mul(zt2[:, :], tt[:, :], tt[:, :])
                for kl in range(F_T // CHUNK):
                    k = t * (F_T // CHUNK) + kl
                    sl = slice(kl * CHUNK, (kl + 1) * CHUNK)
                    for ps, src_ in ((ps_t, tt), (ps_t2, zt2), (ps_d2, zd2)):
                        nc.tensor.matmul(
                            ps[:NCH, :],
                            lhsT=oneh[:, k, :],
                            rhs=src_[:, sl],
                            start=(k == 0),
                            stop=(k == NCH - 1),
                        )
            res_t = cpool.tile([128, CHUNK], F32, tag="res_t")
            res_t2 = cpool.tile([128, CHUNK], F32, tag="res_t2")
            res_d2 = cpool.tile([128, CHUNK], F32, tag="res_d2")
            nc.vector.tensor_copy(res_t[:NCH, :], ps_t[:NCH, :])
            nc.vector.tensor_copy(res_t2[:NCH, :], ps_t2[:NCH, :])
            nc.vector.tensor_copy(res_d2[:NCH, :], ps_d2[:NCH, :])
            nc.sync.dma_start(pt[:, :], res_t[:NCH, :])
            nc.sync.dma_start(pt2[:, :], res_t2[:NCH, :])
            nc.sync.dma_start(pd2[:, :], res_d2[:NCH, :])
    nc.compile()
    return nc


def _get_nc():
    if "nc" not in _cache:
        _cache["nc"] = _build()
    return _cache["nc"]


def _prep(y_pred, y_true, basin):
    """Counting-sort by basin, zero-pad runs to multiples of G, column layout."""
    yp = np.asarray(y_pred, dtype=np.float32).ravel()
    yt = np.asarray(y_true, dtype=np.float32).ravel()
    b = np.asarray(basin).ravel().astype(np.uint16)
    counts = np.bincount(b, minlength=N_BASINS).astype(np.int64)
    grp = -(counts // -G)  # ceil(counts/G) groups per basin
    gstart = np.zeros(N_BASINS + 1, np.int64)
    np.cumsum(grp, out=gstart[1:])
    n_used = int(gstart[-1])
    assert n_used * G <= E_TOT
    cstart = np.zeros(N_BASINS + 1, np.int64)
    np.cumsum(counts, out=cstart[1:])
    order = np.argsort(b, kind="stable")
    b_sorted = b[order]
    # padded-stream rank
    r = gstart[b_sorted] * G + (np.arange(N_TOTAL, dtype=np.int64) - cstart[b_sorted])
    # rank -> (core, tile, partition, column) -> DRAM index
    i = r % 128
    q = r // 128
    c = q // COLS
    q_core = q % COLS
    t = q_core // F_T
    col = q_core % F_T
    pos = c * E + t * (128 * F_T) + i * F_T + col
    bf = ml_dtypes.bfloat16
    yt_pad = np.zeros(E_TOT, bf)
    yp_pad = np.zeros(E_TOT, bf)
    yt_pad[pos] = yt[order]
    yp_pad[pos] = yp[order]
    return yt_pad, yp_pad, counts, grp, n_used


def _in_maps(yt_pad, yp_pad):
    return [
        {"yt": yt_pad[c * E : (c + 1) * E], "yp": yp_pad[c * E : (c + 1) * E]}
        for c in range(N_CORES)
    ]


def _finish(results, counts, grp, n_used):
    """Combine per-group device partials into the NSE mean (float64)."""

    # result [k, n] flattens to within-core group index q_core = k*CHUNK+n
    def order_groups(name):
        return np.concatenate(
            [
                np.asarray(results[c][name]).astype(np.float64).reshape(-1)
                for c in range(N_CORES)
            ]
        )[:n_used]

    gb = np.repeat(np.arange(N_BASINS), grp)
    s_t = np.bincount(gb, weights=order_groups("pt"), minlength=N_BASINS)
    s_t2 = np.bincount(gb, weights=order_groups("pt2"), minlength=N_BASINS)
    ss_res = np.bincount(gb, weights=order_groups("pd2"), minlength=N_BASINS)
    cnt = counts.astype(np.float64)
    ss_tot = s_t2 - s_t * s_t / cnt
    nse = 1.0 - ss_res / (ss_tot + EPS)
    return np.float32(nse.mean())


def kernel(y_pred, y_true, basin):
    yt_pad, yp_pad, counts, grp, n_used = _prep(y_pred, y_true, basin)
    nc = _get_nc()
    res = run_bass_kernel_spmd(nc, _in_maps(yt_pad, yp_pad), list(range(N_CORES)))
    return _finish(res.results, counts, grp, n_used)
